# revision 21
# baseline (speedup 1.0000x reference)
"""Causal self-attention (B=8, T=2048, C=256, H=8, D=32) on 8 trn2 NeuronCores.

Sharding: pure data-parallel over batch — core b computes batch element b
end-to-end (no collectives).

v2 redesign (engine-balance driven; ACT exp ~137us is the floor):
  - x is DMA-transposed (fp32, XBAR) straight from DRAM into SBUF per
    512-col chunk, then cast to bf16: no PE transposes, minimal DVE.
  - qkv + attention are interleaved per m-chunk so the ACT pipeline
    starts early and the PE never sits idle long (HAM clock gate).
  - Softmax normalization: denominator rows (from the v_aug ones-column
    PV trick) are reciprocal'd with reciprocal_approx_fast ([1,W] strips,
    ~5x faster than InstReciprocal), then broadcast across partitions
    with ONE K=2 fp32 indicator matmul per (mc,g,b) instead of hi/lo
    bf16 ones-matmul pairs; the normalize multiply reads both operands
    from PSUM.
  - Attention layout unchanged: S^T tiles [128 keys, 4 heads x W],
    single big ACT exp per (mc,g,nn), triangular bf16 mask mul on DVE
    for diagonal blocks, PV with M=33 (32 v rows + ones) col-packed.
"""

import numpy as np
from contextlib import ExitStack

import concourse.bass as bass
import concourse.bacc as bacc
import concourse.mybir as mybir
import concourse.tile as tile
from concourse.bass import ds
from concourse.bass_utils import run_bass_kernel_spmd
from concourse.masks import make_upper_triangular

FP32 = mybir.dt.float32
BF16 = mybir.dt.bfloat16

C = 256
H = 8
D = 32
N_CORES = 8
SCALE = 1.0 / float(np.sqrt(np.float32(D)))


def build_body(ctx: ExitStack, tc: tile.TileContext, x, wa, wp, out, T: int):
    nc = tc.nc
    TT = T // 128              # number of 128-row t-tiles
    W = min(512, T)            # m-chunk width
    MCN = T // W               # number of m-chunks
    WT = W // 128              # n-tiles (t-tiles) per m-chunk

    const = ctx.enter_context(tc.tile_pool(name="const", bufs=1))
    wpool = ctx.enter_context(tc.tile_pool(name="wpool", bufs=1))
    wstage = ctx.enter_context(tc.tile_pool(name="wstage", bufs=4))
    # fresh slot per transpose-DMA chunk: keeps HWDGE input DMAs at zero
    # sem waits (walrus rejects HWDGE DMAs with >1 sem wait)
    xTstg = ctx.enter_context(tc.tile_pool(name="xTstg", bufs=2 * MCN))
    xTp = ctx.enter_context(tc.tile_pool(name="xTp", bufs=1))
    qkTp = ctx.enter_context(tc.tile_pool(name="qkTp", bufs=1))
    vaugp = ctx.enter_context(tc.tile_pool(name="vaugp", bufs=TT))
    ptp = ctx.enter_context(tc.tile_pool(name="ptp", bufs=3))
    ytp = ctx.enter_context(tc.tile_pool(name="ytp", bufs=1))
    nrmp = ctx.enter_context(tc.tile_pool(name="nrmp", bufs=4))
    ostp = ctx.enter_context(tc.tile_pool(name="ostp", bufs=4))

    # PSUM budget (8 banks of [128, 2KB]):
    #   ps_s: 1 tag x 1 buf x [128, 4W] fp32 = 4 banks
    #   ps_y: 2 tags x 1 buf x [128, W] fp32 = 2 banks
    #   ps_sm: 1 shared tag x 2 bufs x <=1 bank = 2 banks
    ps_s = ctx.enter_context(tc.tile_pool(name="ps_s", bufs=2, space="PSUM"))
    ps_y = ctx.enter_context(tc.tile_pool(name="ps_y", bufs=1, space="PSUM"))
    ps_q = ctx.enter_context(tc.tile_pool(name="ps_q", bufs=1, space="PSUM"))
    ps_n = ctx.enter_context(tc.tile_pool(name="ps_n", bufs=1, space="PSUM"))

    # Pre-place ONE activation-table load for set 6
    # (natural_log_exp_and_others: has Exp AND Ln) so the Bacc fixpoint
    # pass doesn't thrash exp_and_others <-> natural_log per (mc,g)
    # (25 table loads x ~2.7us observed).
    nc.scalar.add_instruction(mybir.InstLoadActFuncSet(
        name=nc.get_next_instruction_name(), ins=[], outs=[],
        act_func_set_id=6))

    # --- constants: triangular keep-mask (n' <= m'), replicated 4x ---
    tri = const.tile([128, 128], BF16)
    make_upper_triangular(nc, tri[:], val=1.0, diag=True)
    tri4 = const.tile([128, 512], BF16)
    for i in range(4):
        nc.vector.tensor_copy(tri4[:, 128 * i:128 * (i + 1)], tri[:])
    tri4v = tri4[:].rearrange("p (h m) -> p h m", h=4)
    # fp32 ones row [1, D] for the K=1 denominator broadcast matmuls
    # (built via bf16 memset + cast — fp32 memset trips the BIR verifier)
    ones_bf = const.tile([1, D], BF16)
    nc.gpsimd.memset(ones_bf[:], 1.0)
    ones_f = const.tile([1, D], FP32)
    nc.vector.tensor_copy(ones_f[:], ones_bf[:])
    # identity for PE transposes
    ident = const.tile([128, 128], BF16)
    from concourse.masks import make_identity
    make_identity(nc, ident[:])

    # --- weights: load fp32, cast to bf16 ---
    wa_bf = []
    wp_bf = []
    for k in range(2):
        wa_f = wstage.tile([128, 3 * C], FP32, name=f"wa_f{k}", tag="wstage")
        nc.sync.dma_start(wa_f[:], wa[128 * k:128 * (k + 1), :])
        wab = wpool.tile([128, 3 * C], BF16, name=f"wa_bf{k}")
        nc.vector.tensor_copy(wab[:], wa_f[:])
        wa_bf.append(wab)
        wp_f = wstage.tile([128, C], FP32, name=f"wp_f{k}", tag="wstage")
        nc.sync.dma_start(wp_f[:], wp[128 * k:128 * (k + 1), :])
        wpb = wpool.tile([128, C], BF16, name=f"wp_bf{k}")
        nc.vector.tensor_copy(wpb[:], wp_f[:])
        wp_bf.append(wpb)

    xT = [xTp.tile([128, T], BF16, name=f"xT{k}") for k in range(2)]
    qkT = [qkTp.tile([128, T], BF16, name=f"qkT{f}") for f in range(4)]
    vaug = [None] * TT
    yT = [ytp.tile([128, T], BF16, name=f"yT{g}") for g in range(2)]

    def qkv_chunk(tck):
        """x load+transpose+cast, then q/k (transposed) + v_aug for chunk."""
        cs = slice(W * tck, W * (tck + 1))
        for tt in range(WT * tck, WT * (tck + 1)):
            x_f = xTstg.tile([128, C], FP32, name="x_f")
            nc.sync.dma_start(x_f[:], x[128 * tt:128 * (tt + 1), :])
            x_bf = xTstg.tile([128, C], BF16, name="x_bf")
            nc.vector.tensor_copy(x_bf[:], x_f[:])
            for k in range(2):
                tp_ps = ps_q.tile([128, 128], BF16, name="tp_ps", tag="q")
                nc.tensor.transpose(tp_ps[:], x_bf[:, 128 * k:128 * (k + 1)],
                                    ident[:])
                nc.vector.tensor_copy(xT[k][:, 128 * tt:128 * (tt + 1)],
                                      tp_ps[:])
        for f in range(4):
            ps = ps_q.tile([128, W], FP32, name="qk_ps", tag="q")
            for k in range(2):
                nc.tensor.matmul(
                    ps[:],
                    wa_bf[k][:, 128 * f:128 * (f + 1)],
                    xT[k][:, cs],
                    start=(k == 0),
                    stop=(k == 1),
                )
            nc.vector.tensor_copy(qkT[f][:, cs], ps[:])
        for tt in range(WT * tck, WT * (tck + 1)):
            ps = ps_q.tile([128, C], FP32, name="v_ps", tag="q")
            for k in range(2):
                nc.tensor.matmul(
                    ps[:],
                    xT[k][:, 128 * tt:128 * (tt + 1)],
                    wa_bf[k][:, 2 * C:3 * C],
                    start=(k == 0),
                    stop=(k == 1),
                )
            va = vaugp.tile([128, H * (D + 1)], BF16, name="va")
            nc.gpsimd.memset(va[:], 1.0)
            nc.vector.tensor_copy(
                va[:].rearrange("p (h d) -> p h d", h=H)[:, :, 0:D],
                ps[:].rearrange("p (h d) -> p h d", h=H),
            )
            vaug[tt] = va

    def attn_chunk(mc):
        for g in range(2):
            qt = qkT[g]       # q features for heads 4g..4g+3
            kt = qkT[2 + g]   # k features
            nn_count = WT * (mc + 1)
            nn_last = nn_count - 1
            # one [128, 2W] tile: b=0 in cols 0..W, b=1 in cols W..2W.
            # rows: 64i..64i+31 = y (head 2b+i), 64i+32 = denominator.
            y2 = ps_y.tile([128, 2 * W], FP32, name="y2")
            for nn in range(nn_count):
                j = nn - WT * mc  # >= 0 on diagonal-crossing tiles
                off = max(0, 128 * j)
                # head-PAIR granularity: [128, 2W] score tiles (2 PSUM
                # banks) double-buffered so exp(pair A) overlaps the S
                # matmuls of pair B; pair p owns PE row-quadrants
                # 64p..64p+63 so both in-flight pairs stream concurrently.
                for p in range(2):
                    s_ps = ps_s.tile([128, 2 * W], FP32, name="s_ps")
                    for hh in range(2):
                        h4 = 2 * p + hh
                        nc.tensor.matmul(
                            s_ps[:, W * hh:W * (hh + 1)],
                            kt[32 * h4:32 * (h4 + 1),
                               128 * nn:128 * (nn + 1)],
                            qt[32 * h4:32 * (h4 + 1),
                               W * mc:W * (mc + 1)],
                            start=True,
                            stop=True,
                            tile_position=(64 * p + 32 * hh, 0),
                            skip_group_check=True,
                        )
                    pt = ptp.tile([128, 2 * W], BF16, name="pt")
                    sv = s_ps[:].rearrange("p (h m) -> p h m", h=2)
                    pv = pt[:].rearrange("p (h m) -> p h m", h=2)
                    nc.scalar.activation(
                        pv[:, :, ds(off, W - off)],
                        sv[:, :, ds(off, W - off)],
                        mybir.ActivationFunctionType.Exp,
                        scale=SCALE,
                    )
                    if j >= 0:
                        # triangular mask on each head's diag block
                        blk = pv[:, :, ds(off, 128)]
                        nc.gpsimd.tensor_mul(blk, blk, tri4v[:, 0:2, 0:128])
                    for i in range(2):
                        hg = 4 * g + 2 * p + i
                        nc.tensor.matmul(
                            y2[ds(64 * i, D + 1), ds(W * p + off, W - off)],
                            vaug[nn][:, (D + 1) * hg:(D + 1) * hg + (D + 1)],
                            pt[:, W * i + off:W * (i + 1)],
                            start=(nn == 0),
                            stop=(nn == nn_last),
                            tile_position=(0, 64 * i),
                            skip_group_check=True,
                        )
            # normalize: yT[g][64b+32i : +32, mc cols] = y * recip(denom).
            # recip on ACT as exp(-ln(x)) (~1e-6 rel; ln+exp share one
            # table set with the softmax exp, so no table switches).
            # PSUM reads from partition offset>0 may span at most 32
            # partitions: two [32, 2W] Ln calls (offsets 32 / 96) cover
            # all 4 denom strips of this g; one [64, 2W] Exp finishes.
            # Broadcast with one K=1 bf16 ones matmul per head.
            lnt = nrmp.tile([64, 2 * W], FP32, name="lnt")
            for i in range(2):
                nc.scalar.activation(lnt[ds(32 * i, D), :],
                                     y2[ds(64 * i + D, D), :],
                                     mybir.ActivationFunctionType.Ln)
            # separate [1, 2W] exp outputs per i: the broadcast matmul
            # needs lhsT and rhs to start at the same partition (0)
            rr = []
            for i in range(2):
                r = nrmp.tile([1, 2 * W], BF16, name=f"rr{i}")
                nc.scalar.activation(r[:], lnt[ds(32 * i, 1), :],
                                     mybir.ActivationFunctionType.Exp,
                                     scale=-1.0)
                rr.append(r)
            for b in range(2):
                bc_ps = ps_n.tile([128, W], FP32, name="bc_ps", tag="n")
                for i in range(2):
                    # tile_position must match the PSUM partition offset
                    nc.tensor.matmul(bc_ps[ds(64 * i, D), :], ones_bf[:],
                                     rr[i][0:1, ds(W * b, W)],
                                     start=True, stop=True,
                                     tile_position=(0, 64 * i),
                                     skip_group_check=True)
                # tensor_tensor may read only ONE operand from PSUM:
                # stage the broadcast in SBUF (one copy for both i)
                bcast = nrmp.tile([96, W], FP32, name="bcast")
                nc.vector.tensor_copy(bcast[:], bc_ps[0:96, :])
                for i in range(2):
                    nc.vector.tensor_mul(
                        yT[g][64 * b + 32 * i:64 * b + 32 * i + 32,
                              W * mc:W * (mc + 1)],
                        y2[ds(64 * i, D), ds(W * b, W)],
                        bcast[ds(64 * i, D), :],
                    )
        # projection for this m-chunk's t-tiles
        for tt in range(WT * mc, WT * (mc + 1)):
            ps = ps_n.tile([128, C], FP32, name="pj_ps", tag="n")
            for g in range(2):
                nc.tensor.matmul(
                    ps[:],
                    yT[g][:, 128 * tt:128 * (tt + 1)],
                    wp_bf[g][:],
                    start=(g == 0),
                    stop=(g == 1),
                )
            ost = ostp.tile([128, C], FP32, name="ost")
            nc.vector.tensor_copy(ost[:], ps[:])
            # SWDGE: out-store waits are executed by Q7 ucode (no 1-wait cap)
            nc.gpsimd.dma_start(out[128 * tt:128 * (tt + 1), :], ost[:])

    # one chunk of lookahead: during attn(mc)'s normalize/proj tail the
    # engines can pull already-emitted qkv(mc+1) work instead of idling
    qkv_chunk(0)
    for mc in range(MCN):
        if mc + 1 < MCN:
            qkv_chunk(mc + 1)
        attn_chunk(mc)


def build_nc(T: int = 2048) -> bass.Bass:
    # Bacc (not raw Bass): its compile() pass legalizes multi-sem waits via
    # event semaphores — walrus only accepts one sem wait per instruction.
    nc = bacc.Bacc("TRN2", target_bir_lowering=False, debug=False,
                   num_devices=N_CORES)
    x_d = nc.dram_tensor("x", [T, C], FP32, kind="ExternalInput")
    wa_d = nc.dram_tensor("w_attn", [C, 3 * C], FP32, kind="ExternalInput")
    wp_d = nc.dram_tensor("w_proj", [C, C], FP32, kind="ExternalInput")
    out_d = nc.dram_tensor("out", [T, C], FP32, kind="ExternalOutput")
    with tile.TileContext(nc) as tc:
        with ExitStack() as ctx:
            build_body(ctx, tc, x_d.ap(), wa_d.ap(), wp_d.ap(), out_d.ap(), T)
    nc.compile()
    return nc


_NC_CACHE: dict[int, bass.Bass] = {}


def _get_nc(T: int) -> bass.Bass:
    if T not in _NC_CACHE:
        _NC_CACHE[T] = build_nc(T)
    return _NC_CACHE[T]


def kernel(x: np.ndarray, w_attn: np.ndarray, w_proj: np.ndarray,
           **run_kwargs) -> np.ndarray:
    B, T, C_ = x.shape
    assert B == N_CORES and C_ == C
    nc = _get_nc(T)
    wa = np.ascontiguousarray(w_attn, dtype=np.float32)
    wp = np.ascontiguousarray(w_proj, dtype=np.float32)
    in_maps = [
        {"x": np.ascontiguousarray(x[b], dtype=np.float32), "w_attn": wa,
         "w_proj": wp}
        for b in range(B)
    ]
    res = run_bass_kernel_spmd(nc, in_maps, list(range(N_CORES)), **run_kwargs)
    out = np.stack([res.results[b]["out"] for b in range(B)])
    return out.astype(np.float32)
